# revision 11
# baseline (speedup 1.0000x reference)
"""FLGC (fused learned group conv) forward for Trainium2, 8-core data parallel.

The reference collapses to:  out[b, j, hw] = sum_c W[j, c] * x[b, c, hw]
where W folds the softmax gates, group mask, s/t gains, and the double
output permutation:
    W = (conv[:,:,0,0] * t_gain[:,None] * mask * s_gain[None,:])[p[p], :]

W is group-block-sparse: row j only reads channels c with s[c] == t[p[p][j]].
On the host we pick ONE ordering of the 16 groups (searched to minimize
work), sort input channels and output channels by it, and cut both sides
into four dense 128-channel blocks. In that ordering the sorted W is banded:
only ~6 of the 16 (in-block, out-block) weight blocks are nonzero.

The device kernel is a dense blocked matmul in FP16 that skips the zero
blocks. The problem is HBM-bandwidth bound (headroom target 'memory'), so
the host pre-tiles x into a layout where every device load is ONE fully
contiguous (128 x KB*N_DMA) fp16 DMA (~3 MB), and every store is likewise
one contiguous DMA — minimal descriptor overhead, near-peak HBM bandwidth,
and half the bytes of fp32. PSUM accumulates in fp32; the PSUM->SBUF
copy casts to fp16 and alternates Vector/Scalar engines so neither becomes
the bottleneck. Host casts the fp16 result back to fp32 (max rel err ~1e-3,
well under the 2e-2 gate). Batch is sharded 2 images per core across 8.
"""

import os

import numpy as np

import concourse.bacc as bacc
import concourse.bass as bass
import concourse.mybir as mybir
import concourse.tile as tile
from concourse.bass import ds
from concourse.bass_utils import run_bass_kernel_spmd

# NTFF tracing is not reachable through the axon tunnel in this container
# (antenv.axon_hooks absent); a stray BASS_TRACE=1 would crash the run.
os.environ["BASS_NEVER_TRACE"] = "1"

# Problem shapes (hardcoded per harness contract)
B, C, H, W_SP = 16, 512, 96, 96
G = 16
HW = H * W_SP            # 9216
N_CORES = 8
B_LOC = B // N_CORES     # 2
KB = C // 128            # 4 input-channel blocks
MB = C // 128            # 4 output-channel blocks
N_MM = 512               # spatial columns per matmul (one fp32 PSUM bank)

# Tunables (fixed after on-HW A/B)
N_DMA = 3072             # spatial columns per DMA tile (one contiguous 3MB DMA)
OUT_ENG = "scalar"       # engine issuing output DMAs: sync | scalar | gpsimd
                         # (scalar = ACT HWDGE ring, disjoint from the SP ring
                         # issuing input DMAs — measured ~4% faster than sync)
COPY_SPLIT = True        # alternate PSUM->SBUF copies between DVE and ACT

F16 = mybir.dt.float16
F32 = mybir.dt.float32

LAST_RESULT = None       # BassKernelResults of the most recent run (for test.py)
_NC_CACHE = {}


def _build_nc_banded(
    pairs,
    n_dma=N_DMA,
    repeat=1,
    loop=False,
    out_eng=OUT_ENG,
    copy_split=COPY_SPLIT,
    dma_only=False,
    x_bufs=3,
    o_bufs=3,
    dve_frac=(1, 2),
    unroll=1,
):
    """pairs: ordered tuple of (in_block, out_block) nonzero weight blocks.
    x/y are in band-sorted channel order and host-pre-tiled so each (b, t)
    load/store is one fully contiguous DMA. wt[p] is the lhsT (k, m)
    128x128 fp16 block for pairs[p]."""
    np_ = len(pairs)
    sub_n = n_dma // N_MM
    nt = HW // n_dma
    assert n_dma * nt == HW and sub_n * N_MM == n_dma

    nc = bacc.Bacc("TRN2", target_bir_lowering=False, debug=False)
    x_d = nc.dram_tensor("x", (B_LOC, nt, 128, KB, n_dma), F16, kind="ExternalInput")
    wt_d = nc.dram_tensor("wt", (np_, 128, 128), F16, kind="ExternalInput")
    y_d = nc.dram_tensor("y", (B_LOC, nt, 128, MB, n_dma), F16, kind="ExternalOutput")

    by_out = [[] for _ in range(MB)]
    for idx, (i, j) in enumerate(pairs):
        by_out[j].append((idx, i))

    with tile.TileContext(nc) as tc:
        with (
            tc.tile_pool(name="wt", bufs=1) as wt_pool,
            tc.tile_pool(name="xin", bufs=x_bufs) as x_pool,
            tc.tile_pool(name="out", bufs=o_bufs) as o_pool,
            tc.tile_pool(name="ps", bufs=8, space=bass.MemorySpace.PSUM) as ps_pool,
        ):
            wt_sb = wt_pool.tile([128, np_, 128], F16)
            for p in range(np_):
                nc.sync.dma_start(wt_sb[:, p, :], wt_d[p])

            def body():
                for b in range(B_LOC):
                    for t in range(nt):
                        out_engine = {
                            "sync": nc.sync,
                            "scalar": nc.scalar,
                            "gpsimd": nc.gpsimd,
                            "alt": nc.sync if (b * nt + t) % 2 else nc.scalar,
                        }[out_eng]
                        x_sb = x_pool.tile([128, KB, n_dma], F16, tag="x_sb")
                        in_engine = (
                            nc.scalar
                            if (out_eng == "alt" and (b * nt + t) % 2)
                            else nc.sync
                        )
                        in_engine.dma_start(x_sb[:], x_d[b, t])
                        if dma_only:
                            out_engine.dma_start(y_d[b, t], x_sb[:])
                            continue
                        o_sb = o_pool.tile([128, MB, n_dma], F16, tag="o_sb")
                        ci = 0
                        for sub in range(sub_n):
                            for m0 in range(MB):
                                ps = ps_pool.tile([128, N_MM], F32, tag="ps")
                                blocks = by_out[m0]
                                for n, (idx, i) in enumerate(blocks):
                                    nc.tensor.matmul(
                                        ps[:],
                                        wt_sb[:, idx, :],
                                        x_sb[:, i, ds(sub * N_MM, N_MM)],
                                        start=(n == 0),
                                        stop=(n == len(blocks) - 1),
                                    )
                                dst = o_sb[:, m0, ds(sub * N_MM, N_MM)]
                                if copy_split and (ci % dve_frac[1]) >= dve_frac[0]:
                                    nc.scalar.copy(dst, ps[:])
                                else:
                                    nc.vector.tensor_copy(dst, ps[:])
                                ci += 1
                        out_engine.dma_start(y_d[b, t], o_sb[:])

            if loop:
                with tc.For_i(0, repeat, 1):
                    for _ in range(unroll):
                        body()
            else:
                for _ in range(repeat * unroll):
                    body()
    nc.compile()
    return nc


def _softmax(a):
    a = a - a.max(axis=1, keepdims=True)
    e = np.exp(a)
    return e / e.sum(axis=1, keepdims=True)


def _gates(conv, S, T):
    """Replicate the reference's gate math; return folded W plus group ids."""
    s_hat = _softmax(S.astype(np.float32))
    t_hat = _softmax(T.astype(np.float32))
    s = s_hat.argmax(axis=1)
    t = t_hat.argmax(axis=1)
    c_in, c_out = S.shape[0], T.shape[0]
    s_gain = s_hat[np.arange(c_in), s]
    t_gain = t_hat[np.arange(c_out), t]
    mask = (t[:, None] == s[None, :]).astype(np.float32)
    w_eff = conv[:, :, 0, 0] * t_gain[:, None] * mask
    p = np.argsort(t, kind="stable")
    pp = p[p]
    W = (w_eff * s_gain[None, :])[pp, :].astype(np.float32)
    gfin = t[pp]  # group id of each final output channel
    return W, s, gfin


def _count_pairs(order, ins, outs):
    pairs = set()
    icum = ocum = 0
    for g in order:
        if ins[g] or outs[g]:
            i0 = icum // 128
            i1 = (icum + max(ins[g], 1) - 1) // 128
            o0 = ocum // 128
            o1 = (ocum + max(outs[g], 1) - 1) // 128
            pairs.update(
                (i, o) for i in range(i0, i1 + 1) for o in range(o0, o1 + 1)
            )
        icum += ins[g]
        ocum += outs[g]
    return pairs


def _find_band_order(s, gfin, trials=60000):
    """Search a group ordering minimizing nonzero (in,out) weight blocks."""
    ins = np.bincount(s, minlength=G)
    outs = np.bincount(gfin, minlength=G)
    rng = np.random.default_rng(12345)
    order = np.arange(G)
    best_p, best_o = len(_count_pairs(order, ins, outs)), order.copy()
    for _ in range(trials):
        rng.shuffle(order)
        p = len(_count_pairs(order, ins, outs))
        if p < best_p:
            cur = order.copy()
            improved = True
            while improved:
                improved = False
                for a in range(G):
                    for b_ in range(a + 1, G):
                        cur[a], cur[b_] = cur[b_], cur[a]
                        q = len(_count_pairs(cur, ins, outs))
                        if q < p:
                            p = q
                            improved = True
                        else:
                            cur[a], cur[b_] = cur[b_], cur[a]
            best_p, best_o = p, cur.copy()
        if best_p <= 6:
            break
    return best_o, sorted(_count_pairs(best_o, ins, outs))


def _prep(x, conv, S, T, n_dma=N_DMA):
    """Host-side prep shared by kernel() and test.py's timing path: fold
    gates into W, band-sort channels, cut into 128-blocks, pre-tile x into
    the contiguous fp16 device layout."""
    W, s, gfin = _gates(conv, S, T)
    order, pairs = _find_band_order(s, gfin)
    pairs = tuple(pairs)
    in_order = np.concatenate([np.nonzero(s == g)[0] for g in order])
    out_order = np.concatenate([np.nonzero(gfin == g)[0] for g in order])
    W_sorted = W[np.ix_(out_order, in_order)]

    wt = np.empty((len(pairs), 128, 128), dtype=np.float16)
    for p, (i, j) in enumerate(pairs):
        wt[p] = W_sorted[j * 128 : (j + 1) * 128, i * 128 : (i + 1) * 128].T

    nt = HW // n_dma
    x_pre = x.reshape(B, C, HW)[:, in_order].astype(np.float16)  # (B, C, HW)
    # [b, t, p, k, n] = x_pre[b, k*128+p, t*n_dma+n]  -> each (b,t) slice is
    # one fully contiguous (128, KB*n_dma) DMA source.
    x_t = np.ascontiguousarray(
        x_pre.reshape(B, KB, 128, nt, n_dma).transpose(0, 3, 2, 1, 4)
    )
    in_maps = [
        {"x": x_t[i * B_LOC : (i + 1) * B_LOC], "wt": wt} for i in range(N_CORES)
    ]
    return pairs, in_maps, out_order


def _unprep(results, out_order, n_dma=N_DMA):
    """Invert the device layout: gather per-core fp16 y tiles back to the
    full fp32 (B, C, H, W) output."""
    nt = HW // n_dma
    y_sorted = np.empty((B, C, HW), dtype=np.float32)
    for i, r in enumerate(results):
        # r["y"]: (B_LOC, nt, 128, MB, n_dma) -> (B_LOC, C, HW)
        blk = r["y"].transpose(0, 3, 2, 1, 4).reshape(B_LOC, C, HW)
        y_sorted[i * B_LOC : (i + 1) * B_LOC] = blk.astype(np.float32)
    out = np.empty((B, C, HW), dtype=np.float32)
    out[:, out_order] = y_sorted
    return np.ascontiguousarray(out.reshape(B, C, H, W_SP))


def kernel(x, conv, S, T):
    global LAST_RESULT
    x = np.ascontiguousarray(np.asarray(x, dtype=np.float32))
    conv = np.asarray(conv, dtype=np.float32)
    S = np.asarray(S, dtype=np.float32)
    T = np.asarray(T, dtype=np.float32)

    pairs, in_maps, out_order = _prep(x, conv, S, T)

    key = ("banded16", pairs, N_DMA, OUT_ENG, COPY_SPLIT)
    if key not in _NC_CACHE:
        _NC_CACHE.clear()
        _NC_CACHE[key] = _build_nc_banded(pairs)
    nc = _NC_CACHE[key]

    res = run_bass_kernel_spmd(nc, in_maps, core_ids=list(range(N_CORES)))
    LAST_RESULT = res
    return _unprep(res.results, out_order)


# revision 14
# speedup vs baseline: 1.0810x; 1.0810x over previous
"""FLGC (fused learned group conv) forward for Trainium2, 8-core data parallel.

The reference collapses to:  out[b, j, hw] = sum_c W[j, c] * x[b, c, hw]
where W folds the softmax gates, group mask, s/t gains, and the double
output permutation:
    W = (conv[:,:,0,0] * t_gain[:,None] * mask * s_gain[None,:])[p[p], :]

W is group-block-sparse: row j only reads channels c with s[c] == t[p[p][j]].
On the host we pick ONE ordering of the 16 groups (searched to minimize
work), sort input channels and output channels by it, and cut both sides
into four dense 128-channel blocks. In that ordering the sorted W is banded:
only ~6 of the 16 (in-block, out-block) weight blocks are nonzero.

The device kernel is a dense blocked matmul in FP16 that skips the zero
blocks. The problem is HBM-bandwidth bound (headroom target 'memory'), so
the host pre-tiles x into a layout where every device load is ONE fully
contiguous (128 x KB*N_DMA) fp16 DMA (~3 MB), and every store is likewise
one contiguous DMA — minimal descriptor overhead, near-peak HBM bandwidth,
and half the bytes of fp32. PSUM accumulates in fp32; the PSUM->SBUF
copy casts to fp16 and alternates Vector/Scalar engines so neither becomes
the bottleneck. Host casts the fp16 result back to fp32 (max rel err ~1e-3,
well under the 2e-2 gate). Batch is sharded 2 images per core across 8.
"""

import os

import numpy as np

import concourse.bacc as bacc
import concourse.bass as bass
import concourse.mybir as mybir
import concourse.tile as tile
from concourse.bass import ds
from concourse.bass_utils import run_bass_kernel_spmd

# NTFF tracing is not reachable through the axon tunnel in this container
# (antenv.axon_hooks absent); a stray BASS_TRACE=1 would crash the run.
os.environ["BASS_NEVER_TRACE"] = "1"

# Problem shapes (hardcoded per harness contract)
B, C, H, W_SP = 16, 512, 96, 96
G = 16
HW = H * W_SP            # 9216
N_CORES = 8
B_LOC = B // N_CORES     # 2
KB = C // 128            # 4 input-channel blocks
MB = C // 128            # 4 output-channel blocks
N_MM = 512               # spatial columns per matmul (one fp32 PSUM bank)

# Tunables (fixed after on-HW A/B)
N_DMA = 3072             # spatial columns per DMA tile (one contiguous 3MB DMA)
OUT_ENG = "scalar"       # engine issuing output DMAs: sync | scalar | gpsimd
                         # (scalar = ACT HWDGE ring, disjoint from the SP ring
                         # issuing input DMAs — measured ~4% faster than sync)
COPY_SPLIT = True        # alternate PSUM->SBUF copies between DVE and ACT

F16 = mybir.dt.float16
F32 = mybir.dt.float32

LAST_RESULT = None       # BassKernelResults of the most recent run (for test.py)
_NC_CACHE = {}


def _build_nc_banded(
    pairs,
    n_dma=N_DMA,
    repeat=1,
    loop=False,
    out_eng=OUT_ENG,
    copy_split=COPY_SPLIT,
    dma_only=False,
    dma_free=False,
    x_bufs=3,
    o_bufs=3,
    dve_frac=(1, 2),
    unroll=1,
):
    """pairs: ordered tuple of (in_block, out_block) nonzero weight blocks.
    x/y are in band-sorted channel order and host-pre-tiled so each (b, t)
    load/store is one fully contiguous DMA. wt[p] is the lhsT (k, m)
    128x128 fp16 block for pairs[p]."""
    np_ = len(pairs)
    sub_n = n_dma // N_MM
    nt = HW // n_dma
    assert n_dma * nt == HW and sub_n * N_MM == n_dma

    nc = bacc.Bacc("TRN2", target_bir_lowering=False, debug=False)
    x_d = nc.dram_tensor("x", (B_LOC, nt, 128, KB, n_dma), F16, kind="ExternalInput")
    wt_d = nc.dram_tensor("wt", (np_, 128, 128), F16, kind="ExternalInput")
    y_d = nc.dram_tensor("y", (B_LOC, nt, 128, MB, n_dma), F16, kind="ExternalOutput")

    by_out = [[] for _ in range(MB)]
    for idx, (i, j) in enumerate(pairs):
        by_out[j].append((idx, i))

    with tile.TileContext(nc) as tc:
        with (
            tc.tile_pool(name="wt", bufs=1) as wt_pool,
            tc.tile_pool(name="xin", bufs=x_bufs) as x_pool,
            tc.tile_pool(name="out", bufs=o_bufs) as o_pool,
            tc.tile_pool(name="ps", bufs=8, space=bass.MemorySpace.PSUM) as ps_pool,
        ):
            wt_sb = wt_pool.tile([128, np_, 128], F16)
            for p in range(np_):
                nc.sync.dma_start(wt_sb[:, p, :], wt_d[p])

            dummy = None
            if dma_free:
                # static source for out-DMAs: no dependency on the input
                # stream, so both DMA directions free-run (wire-floor probe)
                dummy = wt_pool.tile([128, MB, n_dma], F16, tag="dummy")
                nc.vector.memset(dummy[:], 0.0)

            def body():
                for b in range(B_LOC):
                    for t in range(nt):
                        out_engine = {
                            "sync": nc.sync,
                            "scalar": nc.scalar,
                            "gpsimd": nc.gpsimd,
                            "alt": nc.sync if (b * nt + t) % 2 else nc.scalar,
                        }[out_eng]
                        x_sb = x_pool.tile([128, KB, n_dma], F16, tag="x_sb")
                        in_engine = (
                            nc.scalar
                            if (out_eng == "alt" and (b * nt + t) % 2)
                            else nc.sync
                        )
                        in_engine.dma_start(x_sb[:], x_d[b, t])
                        if dma_free:
                            out_engine.dma_start(y_d[b, t], dummy[:])
                            continue
                        if dma_only:
                            out_engine.dma_start(y_d[b, t], x_sb[:])
                            continue
                        o_sb = o_pool.tile([128, MB, n_dma], F16, tag="o_sb")
                        ci = 0
                        for sub in range(sub_n):
                            for m0 in range(MB):
                                ps = ps_pool.tile([128, N_MM], F32, tag="ps")
                                blocks = by_out[m0]
                                for n, (idx, i) in enumerate(blocks):
                                    nc.tensor.matmul(
                                        ps[:],
                                        wt_sb[:, idx, :],
                                        x_sb[:, i, ds(sub * N_MM, N_MM)],
                                        start=(n == 0),
                                        stop=(n == len(blocks) - 1),
                                    )
                                dst = o_sb[:, m0, ds(sub * N_MM, N_MM)]
                                if copy_split and (ci % dve_frac[1]) >= dve_frac[0]:
                                    nc.scalar.copy(dst, ps[:])
                                else:
                                    nc.vector.tensor_copy(dst, ps[:])
                                ci += 1
                        out_engine.dma_start(y_d[b, t], o_sb[:])

            if loop:
                with tc.For_i(0, repeat, 1):
                    for _ in range(unroll):
                        body()
            else:
                for _ in range(repeat * unroll):
                    body()
    nc.compile()
    return nc


def _softmax(a):
    a = a - a.max(axis=1, keepdims=True)
    e = np.exp(a)
    return e / e.sum(axis=1, keepdims=True)


def _gates(conv, S, T):
    """Replicate the reference's gate math; return folded W plus group ids."""
    s_hat = _softmax(S.astype(np.float32))
    t_hat = _softmax(T.astype(np.float32))
    s = s_hat.argmax(axis=1)
    t = t_hat.argmax(axis=1)
    c_in, c_out = S.shape[0], T.shape[0]
    s_gain = s_hat[np.arange(c_in), s]
    t_gain = t_hat[np.arange(c_out), t]
    mask = (t[:, None] == s[None, :]).astype(np.float32)
    w_eff = conv[:, :, 0, 0] * t_gain[:, None] * mask
    p = np.argsort(t, kind="stable")
    pp = p[p]
    W = (w_eff * s_gain[None, :])[pp, :].astype(np.float32)
    gfin = t[pp]  # group id of each final output channel
    return W, s, gfin


def _count_pairs(order, ins, outs):
    pairs = set()
    icum = ocum = 0
    for g in order:
        if ins[g] or outs[g]:
            i0 = icum // 128
            i1 = (icum + max(ins[g], 1) - 1) // 128
            o0 = ocum // 128
            o1 = (ocum + max(outs[g], 1) - 1) // 128
            pairs.update(
                (i, o) for i in range(i0, i1 + 1) for o in range(o0, o1 + 1)
            )
        icum += ins[g]
        ocum += outs[g]
    return pairs


def _find_band_order(s, gfin, trials=60000):
    """Search a group ordering minimizing nonzero (in,out) weight blocks."""
    ins = np.bincount(s, minlength=G)
    outs = np.bincount(gfin, minlength=G)
    rng = np.random.default_rng(12345)
    order = np.arange(G)
    best_p, best_o = len(_count_pairs(order, ins, outs)), order.copy()
    for _ in range(trials):
        rng.shuffle(order)
        p = len(_count_pairs(order, ins, outs))
        if p < best_p:
            cur = order.copy()
            improved = True
            while improved:
                improved = False
                for a in range(G):
                    for b_ in range(a + 1, G):
                        cur[a], cur[b_] = cur[b_], cur[a]
                        q = len(_count_pairs(cur, ins, outs))
                        if q < p:
                            p = q
                            improved = True
                        else:
                            cur[a], cur[b_] = cur[b_], cur[a]
            best_p, best_o = p, cur.copy()
        if best_p <= 6:
            break
    return best_o, sorted(_count_pairs(best_o, ins, outs))


def _prep(x, conv, S, T, n_dma=N_DMA):
    """Host-side prep shared by kernel() and test.py's timing path: fold
    gates into W, band-sort channels, cut into 128-blocks, pre-tile x into
    the contiguous fp16 device layout."""
    W, s, gfin = _gates(conv, S, T)
    order, pairs = _find_band_order(s, gfin)
    pairs = tuple(pairs)
    in_order = np.concatenate([np.nonzero(s == g)[0] for g in order])
    out_order = np.concatenate([np.nonzero(gfin == g)[0] for g in order])
    W_sorted = W[np.ix_(out_order, in_order)]

    wt = np.empty((len(pairs), 128, 128), dtype=np.float16)
    for p, (i, j) in enumerate(pairs):
        wt[p] = W_sorted[j * 128 : (j + 1) * 128, i * 128 : (i + 1) * 128].T

    nt = HW // n_dma
    x_pre = x.reshape(B, C, HW)[:, in_order].astype(np.float16)  # (B, C, HW)
    # [b, t, p, k, n] = x_pre[b, k*128+p, t*n_dma+n]  -> each (b,t) slice is
    # one fully contiguous (128, KB*n_dma) DMA source.
    x_t = np.ascontiguousarray(
        x_pre.reshape(B, KB, 128, nt, n_dma).transpose(0, 3, 2, 1, 4)
    )
    in_maps = [
        {"x": x_t[i * B_LOC : (i + 1) * B_LOC], "wt": wt} for i in range(N_CORES)
    ]
    return pairs, in_maps, out_order


def _unprep(results, out_order, n_dma=N_DMA):
    """Invert the device layout: gather per-core fp16 y tiles back to the
    full fp32 (B, C, H, W) output."""
    nt = HW // n_dma
    y_sorted = np.empty((B, C, HW), dtype=np.float32)
    for i, r in enumerate(results):
        # r["y"]: (B_LOC, nt, 128, MB, n_dma) -> (B_LOC, C, HW)
        blk = r["y"].transpose(0, 3, 2, 1, 4).reshape(B_LOC, C, HW)
        y_sorted[i * B_LOC : (i + 1) * B_LOC] = blk.astype(np.float32)
    out = np.empty((B, C, HW), dtype=np.float32)
    out[:, out_order] = y_sorted
    return np.ascontiguousarray(out.reshape(B, C, H, W_SP))


def kernel(x, conv, S, T):
    global LAST_RESULT
    x = np.ascontiguousarray(np.asarray(x, dtype=np.float32))
    conv = np.asarray(conv, dtype=np.float32)
    S = np.asarray(S, dtype=np.float32)
    T = np.asarray(T, dtype=np.float32)

    pairs, in_maps, out_order = _prep(x, conv, S, T)

    key = ("banded16", pairs, N_DMA, OUT_ENG, COPY_SPLIT)
    if key not in _NC_CACHE:
        _NC_CACHE.clear()
        _NC_CACHE[key] = _build_nc_banded(pairs)
    nc = _NC_CACHE[key]

    res = run_bass_kernel_spmd(nc, in_maps, core_ids=list(range(N_CORES)))
    LAST_RESULT = res
    return _unprep(res.results, out_order)


# revision 17
# speedup vs baseline: 1.1159x; 1.0323x over previous
"""FLGC (fused learned group conv) forward for Trainium2, 8-core data parallel.

The reference collapses to:  out[b, j, hw] = sum_c W[j, c] * x[b, c, hw]
where W folds the softmax gates, group mask, s/t gains, and the double
output permutation:
    W = (conv[:,:,0,0] * t_gain[:,None] * mask * s_gain[None,:])[p[p], :]

W is group-block-sparse: row j only reads channels c with s[c] == t[p[p][j]].
On the host we pick ONE ordering of the 16 groups (searched to minimize
work), sort input channels and output channels by it, and cut both sides
into four dense 128-channel blocks. In that ordering the sorted W is banded:
only ~6 of the 16 (in-block, out-block) weight blocks are nonzero.

The device kernel is a dense blocked matmul in FP16 that skips the zero
blocks. The problem is HBM-bandwidth bound (headroom target 'memory'), so
the host pre-tiles x into a layout where every device load is ONE fully
contiguous (128 x KB*N_DMA) fp16 DMA (~3 MB), and every store is likewise
one contiguous DMA — minimal descriptor overhead, near-peak HBM bandwidth,
and half the bytes of fp32. PSUM accumulates in fp32; the PSUM->SBUF
copy casts to fp16 and alternates Vector/Scalar engines so neither becomes
the bottleneck. Host casts the fp16 result back to fp32 (max rel err ~1e-3,
well under the 2e-2 gate). Batch is sharded 2 images per core across 8.
"""

import os

import numpy as np

import concourse.bacc as bacc
import concourse.bass as bass
import concourse.mybir as mybir
import concourse.tile as tile
from concourse.bass import ds
from concourse.bass_utils import run_bass_kernel_spmd

# NTFF tracing is not reachable through the axon tunnel in this container
# (antenv.axon_hooks absent); a stray BASS_TRACE=1 would crash the run.
os.environ["BASS_NEVER_TRACE"] = "1"

# Problem shapes (hardcoded per harness contract)
B, C, H, W_SP = 16, 512, 96, 96
G = 16
HW = H * W_SP            # 9216
N_CORES = 8
B_LOC = B // N_CORES     # 2
KB = C // 128            # 4 input-channel blocks
MB = C // 128            # 4 output-channel blocks
N_MM = 512               # spatial columns per matmul (one fp32 PSUM bank)

# Tunables (fixed after on-HW A/B)
N_DMA = 3072             # spatial columns per DMA tile (one contiguous 3MB DMA)
OUT_ENG = "scalar"       # engine issuing output DMAs: sync | scalar | gpsimd
                         # (scalar = ACT HWDGE ring, disjoint from the SP ring
                         # issuing input DMAs — measured ~4% faster than sync)
COPY_SPLIT = True        # alternate PSUM->SBUF copies between DVE and ACT

F16 = mybir.dt.float16
F32 = mybir.dt.float32

LAST_RESULT = None       # BassKernelResults of the most recent run (for test.py)
_NC_CACHE = {}


def _build_nc_banded(
    pairs,
    n_dma=N_DMA,
    repeat=1,
    loop=False,
    out_eng=OUT_ENG,
    copy_split=COPY_SPLIT,
    dma_only=False,
    dma_free=False,
    x_bufs=3,
    o_bufs=3,
    dve_frac=(1, 2),
    unroll=1,
    split_rings=False,
):
    """pairs: ordered tuple of (in_block, out_block) nonzero weight blocks.
    x/y are in band-sorted channel order and host-pre-tiled so each (b, t)
    load/store is one fully contiguous DMA. wt[p] is the lhsT (k, m)
    128x128 fp16 block for pairs[p]."""
    np_ = len(pairs)
    sub_n = n_dma // N_MM
    nt = HW // n_dma
    assert n_dma * nt == HW and sub_n * N_MM == n_dma

    nc = bacc.Bacc("TRN2", target_bir_lowering=False, debug=False)
    x_d = nc.dram_tensor("x", (B_LOC, nt, 128, KB, n_dma), F16, kind="ExternalInput")
    wt_d = nc.dram_tensor("wt", (np_, 128, 128), F16, kind="ExternalInput")
    y_d = nc.dram_tensor("y", (B_LOC, nt, 128, MB, n_dma), F16, kind="ExternalOutput")

    by_out = [[] for _ in range(MB)]
    for idx, (i, j) in enumerate(pairs):
        by_out[j].append((idx, i))

    with tile.TileContext(nc) as tc:
        with (
            tc.tile_pool(name="wt", bufs=1) as wt_pool,
            tc.tile_pool(name="xin", bufs=x_bufs) as x_pool,
            tc.tile_pool(name="out", bufs=o_bufs) as o_pool,
            tc.tile_pool(name="ps", bufs=8, space=bass.MemorySpace.PSUM) as ps_pool,
        ):
            wt_sb = wt_pool.tile([128, np_, 128], F16)
            for p in range(np_):
                nc.sync.dma_start(wt_sb[:, p, :], wt_d[p])

            dummy = None
            if dma_free:
                # static source for out-DMAs: no dependency on the input
                # stream, so both DMA directions free-run (wire-floor probe)
                dummy = wt_pool.tile([128, MB, n_dma], F16, tag="dummy")
                nc.vector.memset(dummy[:], 0.0)

            def body():
                for b in range(B_LOC):
                    for t in range(nt):
                        out_engine = {
                            "sync": nc.sync,
                            "scalar": nc.scalar,
                            "gpsimd": nc.gpsimd,
                            "alt": nc.sync if (b * nt + t) % 2 else nc.scalar,
                        }[out_eng]
                        x_sb = x_pool.tile([128, KB, n_dma], F16, tag="x_sb")
                        if split_rings:
                            xd = x_d[b, t]
                            nc.sync.dma_start(x_sb[:, 0:2, :], xd[:, 0:2, :])
                            nc.scalar.dma_start(x_sb[:, 2:4, :], xd[:, 2:4, :])
                        else:
                            in_engine = (
                                nc.scalar
                                if (out_eng == "alt" and (b * nt + t) % 2)
                                else nc.sync
                            )
                            in_engine.dma_start(x_sb[:], x_d[b, t])
                        if dma_free:
                            out_engine.dma_start(y_d[b, t], dummy[:])
                            continue
                        if dma_only:
                            out_engine.dma_start(y_d[b, t], x_sb[:])
                            continue
                        o_sb = o_pool.tile([128, MB, n_dma], F16, tag="o_sb")
                        ci = 0
                        for sub in range(sub_n):
                            for m0 in range(MB):
                                ps = ps_pool.tile([128, N_MM], F32, tag="ps")
                                blocks = by_out[m0]
                                for n, (idx, i) in enumerate(blocks):
                                    nc.tensor.matmul(
                                        ps[:],
                                        wt_sb[:, idx, :],
                                        x_sb[:, i, ds(sub * N_MM, N_MM)],
                                        start=(n == 0),
                                        stop=(n == len(blocks) - 1),
                                    )
                                dst = o_sb[:, m0, ds(sub * N_MM, N_MM)]
                                if copy_split and (ci % dve_frac[1]) >= dve_frac[0]:
                                    nc.scalar.copy(dst, ps[:])
                                else:
                                    nc.vector.tensor_copy(dst, ps[:])
                                ci += 1
                        if split_rings:
                            yd = y_d[b, t]
                            nc.scalar.dma_start(yd[:, 0:2, :], o_sb[:, 0:2, :])
                            nc.sync.dma_start(yd[:, 2:4, :], o_sb[:, 2:4, :])
                        else:
                            out_engine.dma_start(y_d[b, t], o_sb[:])

            if loop:
                with tc.For_i(0, repeat, 1):
                    for _ in range(unroll):
                        body()
            else:
                for _ in range(repeat * unroll):
                    body()
    nc.compile()
    return nc


def _softmax(a):
    a = a - a.max(axis=1, keepdims=True)
    e = np.exp(a)
    return e / e.sum(axis=1, keepdims=True)


def _gates(conv, S, T):
    """Replicate the reference's gate math; return folded W plus group ids."""
    s_hat = _softmax(S.astype(np.float32))
    t_hat = _softmax(T.astype(np.float32))
    s = s_hat.argmax(axis=1)
    t = t_hat.argmax(axis=1)
    c_in, c_out = S.shape[0], T.shape[0]
    s_gain = s_hat[np.arange(c_in), s]
    t_gain = t_hat[np.arange(c_out), t]
    mask = (t[:, None] == s[None, :]).astype(np.float32)
    w_eff = conv[:, :, 0, 0] * t_gain[:, None] * mask
    p = np.argsort(t, kind="stable")
    pp = p[p]
    W = (w_eff * s_gain[None, :])[pp, :].astype(np.float32)
    gfin = t[pp]  # group id of each final output channel
    return W, s, gfin


def _count_pairs(order, ins, outs):
    pairs = set()
    icum = ocum = 0
    for g in order:
        if ins[g] or outs[g]:
            i0 = icum // 128
            i1 = (icum + max(ins[g], 1) - 1) // 128
            o0 = ocum // 128
            o1 = (ocum + max(outs[g], 1) - 1) // 128
            pairs.update(
                (i, o) for i in range(i0, i1 + 1) for o in range(o0, o1 + 1)
            )
        icum += ins[g]
        ocum += outs[g]
    return pairs


def _find_band_order(s, gfin, trials=60000):
    """Search a group ordering minimizing nonzero (in,out) weight blocks."""
    ins = np.bincount(s, minlength=G)
    outs = np.bincount(gfin, minlength=G)
    rng = np.random.default_rng(12345)
    order = np.arange(G)
    best_p, best_o = len(_count_pairs(order, ins, outs)), order.copy()
    for _ in range(trials):
        rng.shuffle(order)
        p = len(_count_pairs(order, ins, outs))
        if p < best_p:
            cur = order.copy()
            improved = True
            while improved:
                improved = False
                for a in range(G):
                    for b_ in range(a + 1, G):
                        cur[a], cur[b_] = cur[b_], cur[a]
                        q = len(_count_pairs(cur, ins, outs))
                        if q < p:
                            p = q
                            improved = True
                        else:
                            cur[a], cur[b_] = cur[b_], cur[a]
            best_p, best_o = p, cur.copy()
        if best_p <= 6:
            break
    return best_o, sorted(_count_pairs(best_o, ins, outs))


def _prep(x, conv, S, T, n_dma=N_DMA):
    """Host-side prep shared by kernel() and test.py's timing path: fold
    gates into W, band-sort channels, cut into 128-blocks, pre-tile x into
    the contiguous fp16 device layout."""
    W, s, gfin = _gates(conv, S, T)
    order, pairs = _find_band_order(s, gfin)
    pairs = tuple(pairs)
    in_order = np.concatenate([np.nonzero(s == g)[0] for g in order])
    out_order = np.concatenate([np.nonzero(gfin == g)[0] for g in order])
    W_sorted = W[np.ix_(out_order, in_order)]

    wt = np.empty((len(pairs), 128, 128), dtype=np.float16)
    for p, (i, j) in enumerate(pairs):
        wt[p] = W_sorted[j * 128 : (j + 1) * 128, i * 128 : (i + 1) * 128].T

    nt = HW // n_dma
    x_pre = x.reshape(B, C, HW)[:, in_order].astype(np.float16)  # (B, C, HW)
    # [b, t, p, k, n] = x_pre[b, k*128+p, t*n_dma+n]  -> each (b,t) slice is
    # one fully contiguous (128, KB*n_dma) DMA source.
    x_t = np.ascontiguousarray(
        x_pre.reshape(B, KB, 128, nt, n_dma).transpose(0, 3, 2, 1, 4)
    )
    in_maps = [
        {"x": x_t[i * B_LOC : (i + 1) * B_LOC], "wt": wt} for i in range(N_CORES)
    ]
    return pairs, in_maps, out_order


def _unprep(results, out_order, n_dma=N_DMA):
    """Invert the device layout: gather per-core fp16 y tiles back to the
    full fp32 (B, C, H, W) output."""
    nt = HW // n_dma
    y_sorted = np.empty((B, C, HW), dtype=np.float32)
    for i, r in enumerate(results):
        # r["y"]: (B_LOC, nt, 128, MB, n_dma) -> (B_LOC, C, HW)
        blk = r["y"].transpose(0, 3, 2, 1, 4).reshape(B_LOC, C, HW)
        y_sorted[i * B_LOC : (i + 1) * B_LOC] = blk.astype(np.float32)
    out = np.empty((B, C, HW), dtype=np.float32)
    out[:, out_order] = y_sorted
    return np.ascontiguousarray(out.reshape(B, C, H, W_SP))


def kernel(x, conv, S, T):
    global LAST_RESULT
    x = np.ascontiguousarray(np.asarray(x, dtype=np.float32))
    conv = np.asarray(conv, dtype=np.float32)
    S = np.asarray(S, dtype=np.float32)
    T = np.asarray(T, dtype=np.float32)

    pairs, in_maps, out_order = _prep(x, conv, S, T)

    key = ("banded16", pairs, N_DMA, OUT_ENG, COPY_SPLIT)
    if key not in _NC_CACHE:
        _NC_CACHE.clear()
        _NC_CACHE[key] = _build_nc_banded(pairs)
    nc = _NC_CACHE[key]

    res = run_bass_kernel_spmd(nc, in_maps, core_ids=list(range(N_CORES)))
    LAST_RESULT = res
    return _unprep(res.results, out_order)
